# revision 8
# baseline (speedup 1.0000x reference)
"""Trainium2 Bass kernel for nn_EnergyEwald (gnn_message_passing).

Strategy (8 NeuronCores, SPMD, host combines partials):
  * Real space: pairs are sorted by molecule-of-i on the host, dealt to the 8
    cores, and laid out column-major so every 128-pair SBUF column belongs to
    one molecule.  The device streams Rij components + qi*qj, computes
    erfc(sqrt(a) d)/d with the cutoff mask, reduces each column with a
    ones-matmul on the tensor engine, and returns per-column sums; the host
    segment-sums columns into the 64 molecules.
  * Reciprocal space: per-atom fractional phases u = R @ inv(cell_mol) are
    computed on the host (tiny O(A) work); the device computes the phase
    matrix t = kvec . u with the tensor engine (atoms sharded across cores),
    range-reduces t mod 1 with the magic-number rounding trick on the vector
    engine, evaluates sin/cos on the scalar engine (Sin table is only valid
    on [-pi, pi]), and segment-sums q*cos / q*sin into [mol, kvec]
    accumulators in PSUM via a q-scaled one-hot matmul.  +-k symmetry of the
    kvec lattice halves the transcendental work (checked at runtime, with a
    compiled fallback for asymmetric kvec sets).
  * Host combines the 8 cores' partial sums (the "all-reduce") and applies
    the O(M*K) tail math (q_gauss, k_sq, prefactors, self-interaction).
"""

import math
import os
import sys
from contextlib import ExitStack

import numpy as np

for _p in ("/opt/trn_rl_repo", "/root/.axon_site/_ro/trn_rl_repo"):
    if os.path.isdir(_p) and _p not in sys.path:
        sys.path.insert(0, _p)

import concourse.tile as tile  # noqa: E402
from concourse import bacc, bass_utils, mybir  # noqa: E402

KE = 14.3996
ALPHA = 0.3
CUTOFF = 10.0
SQA = math.sqrt(ALPHA)
FCUT = math.erfc(SQA * CUTOFF) / CUTOFF
TWO_PI = 2.0 * math.pi
MAGIC = 12582912.0  # 1.5 * 2**23: float32 round-to-nearest-integer trick

N_CORES = 8
N_ATOMS = 100000
N_PAIRS = 6400000
N_MOL = 64

APC = N_ATOMS // N_CORES          # atoms per core = 12500
ACH = 98                          # 128-atom chunks per core
APAD = ACH * 128                  # 12544
NSUP = ACH // 2                   # super-chunks of 256 atoms

CT = 1024                         # real-space tile columns
NRT = 7                           # real-space tiles
CCOLS = NRT * CT                  # 7168 columns of 128 pairs per core
PAD_X = 50.0                      # pad pair distance -> masked out

F32 = mybir.dt.float32

_PROG_CACHE = {}


def _build_program(nsets):
    """Build + compile the SPMD device program.

    nsets: number of 512-wide kvec column groups (1 for the symmetric-half
    fast path, 2 for the full-set fallback).
    """
    AluOp = mybir.AluOpType
    AF = mybir.ActivationFunctionType
    NKP = 512 * nsets

    nc = bacc.Bacc("TRN2", target_bir_lowering=False, debug=False,
                   num_devices=N_CORES)

    def din(name, shape):
        return nc.dram_tensor(name, shape, F32, kind="ExternalInput").ap()

    def dout(name, shape):
        return nc.dram_tensor(name, shape, F32, kind="ExternalOutput").ap()

    u_t = din("u_t", [NSUP, 3, 256])       # fractional coords per superchunk
    kv_t = din("kv_t", [3, NKP])           # kvec rows (f32), padded
    qoh = din("qoh", [NSUP, 128, 128])     # q-scaled one-hot, 2 chunks/super
    qcol = din("qcol", [128, ACH])         # q per atom, chunk-major
    xs = din("xs", [128, CCOLS])
    ys = din("ys", [128, CCOLS])
    zs = din("zs", [128, CCOLS])
    qq = din("qq", [128, CCOLS])           # q[i]*q[j] per pair

    o_qr = dout("o_qr", [64, NKP])         # sum q*cos per (mol, kvec)
    o_qi = dout("o_qi", [64, NKP])         # sum q*sin
    o_si = dout("o_si", [64, 1])           # sum q^2 per mol
    o_cs = dout("o_cs", [1, CCOLS])        # per-column pair-potential sums

    with tile.TileContext(nc, trace_sim=False) as tc, ExitStack() as ctx:
        pers = ctx.enter_context(tc.tile_pool(name="pers", bufs=1))
        ra = ctx.enter_context(tc.tile_pool(name="ra", bufs=2))
        io = ctx.enter_context(tc.tile_pool(name="io", bufs=2))
        tmp = ctx.enter_context(tc.tile_pool(name="tmp", bufs=2))
        ps_t = ctx.enter_context(
            tc.tile_pool(name="ps_t", bufs=2 if nsets == 1 else 1,
                         space="PSUM"))
        ps_acc = ctx.enter_context(
            tc.tile_pool(name="ps_acc", bufs=1, space="PSUM"))
        ps_cs = ctx.enter_context(
            tc.tile_pool(name="ps_cs", bufs=1, space="PSUM"))

        # persistent SBUF
        dbuf = pers.tile([128, CCOLS], F32)   # pair distance d
        kv_sb = pers.tile([3, NKP], F32)
        qc_sb = pers.tile([128, ACH], F32)
        ones = pers.tile([128, 1], F32)
        halfpi = pers.tile([128, 1], F32)
        qr_sb = pers.tile([64, NKP], F32)
        qi_sb = pers.tile([64, NKP], F32)
        si_sb = pers.tile([64, 1], F32)

        nc.vector.memset(ones[:], 1.0)
        nc.vector.memset(halfpi[:], math.pi / 2)
        nc.sync.dma_start(kv_sb[:], kv_t[:])
        nc.sync.dma_start(qc_sb[:], qcol[:])

        # ---- Phase R-A: stream pairs, d2 = |Rij|^2, d = sqrt(d2) ----
        for i in range(NRT):
            sl = slice(i * CT, (i + 1) * CT)
            xt = io.tile([128, CT], F32, tag="xt")
            nc.sync.dma_start(xt[:], xs[:, sl])
            yt = io.tile([128, CT], F32, tag="yt")
            nc.sync.dma_start(yt[:], ys[:, sl])
            zt = io.tile([128, CT], F32, tag="zt")
            nc.sync.dma_start(zt[:], zs[:, sl])
            sq = ra.tile([128, CT], F32, tag="sq")
            t1 = ra.tile([128, CT], F32, tag="t1")
            nc.vector.scalar_tensor_tensor(sq[:], xt[:], 1.0, xt[:],
                                           AluOp.mult, AluOp.mult)
            nc.vector.scalar_tensor_tensor(t1[:], yt[:], 1.0, yt[:],
                                           AluOp.mult, AluOp.mult)
            nc.vector.tensor_tensor(sq[:], sq[:], t1[:], AluOp.add)
            nc.vector.scalar_tensor_tensor(t1[:], zt[:], 1.0, zt[:],
                                           AluOp.mult, AluOp.mult)
            nc.vector.tensor_tensor(sq[:], sq[:], t1[:], AluOp.add)
            nc.scalar.activation(dbuf[:, sl], sq[:], AF.Sqrt)

        # ---- Phase R-BC: erf, pair potential, per-column reduce ----
        for i in range(NRT):
            sl = slice(i * CT, (i + 1) * CT)
            et = tmp.tile([128, CT], F32, tag="rnd")
            nc.scalar.activation(et[:], dbuf[:, sl], AF.Erf, scale=-SQA)
            qqt = io.tile([128, CT], F32, tag="qq")
            nc.sync.dma_start(qqt[:], qq[:, sl])
            ri = tmp.tile([128, CT], F32, tag="gneg")
            nc.vector.reciprocal(ri[:], dbuf[:, sl])
            f = tmp.tile([128, CT], F32, tag="ga")
            # (1 - erf(sqa d)) * (1/d)
            nc.vector.scalar_tensor_tensor(f[:], et[:], 1.0, ri[:],
                                           AluOp.add, AluOp.mult)
            p = tmp.tile([128, CT], F32, tag="cs")
            nc.vector.scalar_tensor_tensor(p[:], f[:], FCUT, qqt[:],
                                           AluOp.subtract, AluOp.mult)
            pot = tmp.tile([128, CT], F32, tag="pot")
            nc.vector.scalar_tensor_tensor(pot[:], dbuf[:, sl], CUTOFF, p[:],
                                           AluOp.is_le, AluOp.mult)
            for j in range(CT // 512):
                cps = ps_cs.tile([1, 512], F32, tag="cs")
                nc.tensor.matmul(cps[:], ones[:],
                                 pot[:, j * 512:(j + 1) * 512],
                                 start=True, stop=True)
                cs_sb = io.tile([1, 512], F32, tag="cso")
                nc.vector.tensor_copy(cs_sb[:], cps[:])
                lo = i * CT + j * 512
                nc.sync.dma_start(o_cs[0:1, lo:lo + 512], cs_sb[:])

        # ---- Phase K: reciprocal-space phases + segment sums ----
        qr_ps = ps_acc.tile([64, NKP], F32, tag="qr")
        qi_ps = ps_acc.tile([64, NKP], F32, tag="qi")
        si_ps = ps_acc.tile([64, 1], F32, tag="si")
        for s in range(NSUP):
            qoh_t = io.tile([128, 128], F32, tag="qoh")
            nc.sync.dma_start(qoh_t[:], qoh[s])
            ut = io.tile([3, 256], F32, tag="ut")
            nc.sync.dma_start(ut[:], u_t[s])
            for kset in range(nsets):
                ksl = slice(kset * 512, (kset + 1) * 512)
                tt = ps_t.tile([128, 1024], F32, tag="tt")
                for h in range(2):
                    nc.tensor.matmul(
                        tt[:, h * 512:(h + 1) * 512],
                        ut[:, h * 128:(h + 1) * 128],
                        kv_sb[:, ksl], start=True, stop=True)
                # range reduction: g = round(t) - t in [-0.5, 0.5]
                rnd = tmp.tile([128, 1024], F32, tag="rnd")
                nc.vector.tensor_scalar(rnd[:], tt[:], MAGIC, MAGIC,
                                        AluOp.add, AluOp.subtract)
                gneg = tmp.tile([128, 1024], F32, tag="gneg")
                nc.vector.scalar_tensor_tensor(gneg[:], rnd[:], 0.0, tt[:],
                                               AluOp.add, AluOp.subtract)
                ga = tmp.tile([128, 1024], F32, tag="ga")
                # |g| = max(-g, g)
                nc.vector.scalar_tensor_tensor(ga[:], gneg[:], -1.0, gneg[:],
                                               AluOp.mult, AluOp.max)
                cs_t = tmp.tile([128, 2048], F32, tag="cs")
                # sin(2 pi t) = sin(-2 pi g)
                nc.scalar.activation(cs_t[:, 0:1024], gneg[:], AF.Sin,
                                     scale=-TWO_PI)
                # cos(2 pi t) = sin(pi/2 - 2 pi |g|)
                nc.scalar.activation(cs_t[:, 1024:2048], ga[:], AF.Sin,
                                     scale=-TWO_PI, bias=halfpi[:])
                for h in range(2):
                    ch = 2 * s + h
                    lhs = qoh_t[:, h * 64:(h + 1) * 64]
                    first = (ch == 0)
                    last = (ch == ACH - 1)
                    nc.tensor.matmul(qr_ps[:, ksl], lhs,
                                     cs_t[:, 1024 + h * 512:1536 + h * 512],
                                     start=first, stop=last,
                                     skip_group_check=True)
                    nc.tensor.matmul(qi_ps[:, ksl], lhs,
                                     cs_t[:, h * 512:512 + h * 512],
                                     start=first, stop=last,
                                     skip_group_check=True)
                    if kset == 0:
                        nc.tensor.matmul(si_ps[:], lhs,
                                         qc_sb[:, ch:ch + 1],
                                         start=first, stop=last,
                                         skip_group_check=True)

        # ---- finale: copy accumulators out ----
        nc.vector.tensor_copy(qr_sb[:], qr_ps[:])
        nc.vector.tensor_copy(qi_sb[:], qi_ps[:])
        nc.vector.tensor_copy(si_sb[:], si_ps[:])
        nc.sync.dma_start(o_qr[:], qr_sb[:])
        nc.sync.dma_start(o_qi[:], qi_sb[:])
        nc.sync.dma_start(o_si[:], si_sb[:])

    nc.compile()
    return nc


def _get_program(nsets):
    if nsets not in _PROG_CACHE:
        _PROG_CACHE[nsets] = _build_program(nsets)
    return _PROG_CACHE[nsets]


def _half_kvecs(kvecs):
    """Pick one of each +-k pair.  Returns selected row indices, or None if
    the set is not exactly +-symmetric."""
    nk = kvecs.shape[0]
    key = {tuple(v): i for i, v in enumerate(kvecs)}
    partner = np.full(nk, -1, np.int64)
    for i, v in enumerate(kvecs):
        j = key.get(tuple(-v))
        if j is None:
            return None
        partner[i] = j
    if np.any(partner == np.arange(nk)):
        return None  # self-negative (k=0) unsupported here
    sel = np.where(np.arange(nk) < partner)[0]
    if sel.size * 2 != nk:
        return None
    return sel


def kernel(**inputs):
    q = np.asarray(inputs["partial_charges"], np.float32)[:, 0]
    Rij = np.asarray(inputs["Rij"], np.float32)
    R = np.asarray(inputs["R"], np.float32)
    cell = np.asarray(inputs["cell"], np.float32)
    kvecs = np.asarray(inputs["kvecs"], np.float32)
    idx_m = np.asarray(inputs["idx_m"]).astype(np.int64)
    idx_i = np.asarray(inputs["idx_i"]).astype(np.int64)
    idx_j = np.asarray(inputs["idx_j"]).astype(np.int64)

    sel = _half_kvecs(kvecs)
    if sel is not None:
        kv_use = kvecs[sel]
        wk = 2.0
    else:
        kv_use = kvecs
        wk = 1.0
    nkh = kv_use.shape[0]
    nsets = (nkh + 511) // 512
    NKP = 512 * nsets
    nc = _get_program(nsets)

    # ---------- host prep: reciprocal space ----------
    invc = np.linalg.inv(cell.astype(np.float64))
    u_all = np.einsum("ae,aed->ad", R, invc[idx_m]).astype(np.float32)

    kv_t_np = np.zeros((3, NKP), np.float32)
    kv_t_np[:, :nkh] = kv_use.T

    # ---------- host prep: real space ----------
    mol_pair = idx_m[idx_i].astype(np.int32)
    qq_pair = q[idx_i] * q[idx_j]
    order = np.argsort(mol_pair, kind="stable")
    xs_s = Rij[order, 0]
    ys_s = Rij[order, 1]
    zs_s = Rij[order, 2]
    qq_s = qq_pair[order]
    counts = np.bincount(mol_pair, minlength=N_MOL)
    starts = np.concatenate(([0], np.cumsum(counts)))

    in_maps = []
    colmols = []
    SLOTS = CCOLS * 128
    for c in range(N_CORES):
        gidx = np.full(SLOTS, -1, np.int64)   # [p, col] flattened p*CCOLS+col
        colmol = np.full(CCOLS, -1, np.int32)
        col0 = 0
        for m in range(N_MOL):
            n = counts[m]
            share = (n + N_CORES - 1) // N_CORES
            lo = starts[m] + c * share
            hi = min(starts[m] + n, lo + share)
            ncm = max(hi - lo, 0)
            if ncm == 0:
                continue
            ncols = (ncm + 127) // 128
            js = np.arange(ncm)
            gidx[(js % 128) * CCOLS + col0 + js // 128] = lo + js
            colmol[col0:col0 + ncols] = m
            col0 += ncols
        assert col0 <= CCOLS, f"column overflow: {col0} > {CCOLS}"
        valid = gidx >= 0
        gv = gidx[valid]

        def fill(src, pad):
            a = np.full(SLOTS, pad, np.float32)
            a[valid] = src[gv]
            return a.reshape(128, CCOLS)

        # atoms for this core: round-robin slice keeps mol-sorted order
        a_ids = np.arange(c, N_ATOMS, N_CORES)
        u_core = np.zeros((APAD, 3), np.float32)
        u_core[:APC] = u_all[a_ids]
        q_core = np.zeros(APAD, np.float32)
        q_core[:APC] = q[a_ids]
        m_core = np.zeros(APAD, np.int64)
        m_core[:APC] = idx_m[a_ids]
        qoh_np = np.zeros((APAD, 64), np.float32)
        qoh_np[np.arange(APAD), m_core] = q_core
        qoh_np = qoh_np.reshape(NSUP, 2, 128, 64).transpose(0, 2, 1, 3) \
                       .reshape(NSUP, 128, 128)
        # u_t: [NSUP, 3, 256] — per-superchunk transposed coords
        u_t_np = np.ascontiguousarray(
            u_core.reshape(NSUP, 256, 3).transpose(0, 2, 1))
        qcol_np = np.ascontiguousarray(q_core.reshape(ACH, 128).T)

        in_maps.append({
            "u_t": u_t_np,
            "kv_t": kv_t_np,
            "qoh": np.ascontiguousarray(qoh_np),
            "qcol": qcol_np,
            "xs": fill(xs_s, PAD_X),
            "ys": fill(ys_s, 0.0),
            "zs": fill(zs_s, 0.0),
            "qq": fill(qq_s, 0.0),
        })
        colmols.append(colmol)

    # ---------- run on 8 cores ----------
    res = bass_utils.run_bass_kernel_spmd(nc, in_maps,
                                          core_ids=list(range(N_CORES)))

    # ---------- host combine ----------
    q_real = np.zeros((64, nkh), np.float64)
    q_imag = np.zeros((64, nkh), np.float64)
    self_q2 = np.zeros(64, np.float64)
    y_real = np.zeros(64, np.float64)
    for c in range(N_CORES):
        out = res.results[c]
        q_real += out["o_qr"][:, :nkh]
        q_imag += out["o_qi"][:, :nkh]
        self_q2 += out["o_si"][:, 0]
        cs = out["o_cs"][0]
        cm = colmols[c]
        used = cm >= 0
        y_real += np.bincount(cm[used], weights=cs[used], minlength=64)

    # O(M*K) tail math (float64 on host, cast at the end)
    recip = TWO_PI * np.transpose(invc, (0, 2, 1))        # [M,3,3]
    v_box = np.abs(np.linalg.det(cell.astype(np.float64)))
    prefactor = TWO_PI / v_box
    kv_m = np.einsum("kd,mde->mke", kv_use.astype(np.float64), recip)
    k_sq = np.sum(kv_m ** 2, axis=2)                       # [M,Kh]
    q_gauss = np.exp(-0.25 * k_sq / ALPHA)
    q_dens = q_real ** 2 + q_imag ** 2
    y_ewald = prefactor * np.sum(wk * q_dens * q_gauss / k_sq, axis=1)
    self_int = math.sqrt(ALPHA / math.pi) * self_q2
    y = 0.5 * KE * y_real + KE * (y_ewald - self_int)
    return y.astype(np.float32)
